# revision 28
# baseline (speedup 1.0000x reference)
"""Trainium2 Bass kernel for nn_DingoNet_76854144795142.

Pipeline (per core, 16 of 128 batches, pure data parallel):
  seq -> one-hot (26 symbols) -> conv1d as 3 accumulating PE matmuls from
  per-tap tables G_k = conv_w[:,:,k] @ emb.T  (16-way tile_position packing:
  4 batches x 4 sections concurrently on the 128x128 PE array)
  -> relu -> H' (d-major, section-blocked permutation d' = s*32+c)
  -> T = W_a' @ H' (stationary weights) -> tanh (ACT)
  -> e = v' @ tanhT  (M=1 matmuls, 4-way column packing)
  -> exp folded into PSUM evac with accum_out => softmax sums
  -> ctx = H' @ exp(e) via PE ones-broadcast of exp(e) into PSUM +
     fused tensor_tensor_reduce (mult + free-dim accumulate)
  -> normalization + inverse permutation on host.

Weight-only tables (G_k, permuted W_a / v) are precomputed on host.
"""

import os
import sys

import numpy as np

for _p in ("/opt/trn_rl_repo", "/root/.axon_site/_ro/trn_rl_repo"):
    if os.path.isdir(_p) and _p not in sys.path:
        sys.path.insert(0, _p)

# ---- problem constants (hardcoded per task contract) ----
B = 128          # total batch
L = 8194         # sequence length
PL = 8192        # conv output length
S = 4            # sections
NL = 2048        # positions per section
C = 32           # conv channels
V = 26           # vocab
D = 128          # C * S
NCORES = 8
BPC = B // NCORES      # 16 batches per core
NG = BPC // 4          # 4 groups of 4 batches
CHUNK = 512            # free-dim chunk
NCHUNK = NL // CHUNK   # 4 chunks per section
WIN = CHUNK + 2        # window incl. conv taps

_CACHE = {}


def _build_program():
    """Build + compile the Bass/Tile program once per process."""
    if "prog" in _CACHE:
        return _CACHE["prog"]

    from contextlib import ExitStack

    import concourse.bass as bass
    import concourse.tile as tile
    from concourse import bacc, mybir

    f32 = mybir.dt.float32
    i32 = mybir.dt.int32
    i8 = mybir.dt.int8
    bf16 = mybir.dt.bfloat16
    AF = mybir.ActivationFunctionType
    ALU = mybir.AluOpType

    nc = bacc.Bacc(
        "TRN2",
        target_bir_lowering=False,
        debug=False,
        enable_asserts=True,
        num_devices=NCORES,
    )

    oh_d = nc.dram_tensor("oh", [BPC, 32, L], f32, kind="ExternalInput")
    gt_d = nc.dram_tensor("gt", [128, 3, 32], f32, kind="ExternalInput")
    wat_d = nc.dram_tensor("wat", [128, 128], f32, kind="ExternalInput")
    vcol_d = nc.dram_tensor("vcol", [128, 32], f32, kind="ExternalInput")
    ctxu_d = nc.dram_tensor("ctxu", [128, BPC], f32, kind="ExternalOutput")
    edram = nc.dram_tensor("edram", [BPC, NL], f32, kind="Internal")
    zsum_d = nc.dram_tensor("zsum", [BPC, 1], f32, kind="ExternalOutput")

    with tile.TileContext(nc) as tc, ExitStack() as ctx:
        consts = ctx.enter_context(tc.tile_pool(name="consts", bufs=1))
        ohp = ctx.enter_context(tc.tile_pool(name="ohp", bufs=3))
        hpool = ctx.enter_context(tc.tile_pool(name="hpool", bufs=3))
        ttpool = ctx.enter_context(tc.tile_pool(name="ttpool", bufs=3))
        eepool = ctx.enter_context(tc.tile_pool(name="eepool", bufs=2))
        smallp = ctx.enter_context(tc.tile_pool(name="smallp", bufs=2))
        scrp = ctx.enter_context(tc.tile_pool(name="scrp", bufs=2))
        psum_y = ctx.enter_context(tc.tile_pool(name="psum_y", bufs=1, space="PSUM"))
        psum_t = ctx.enter_context(tc.tile_pool(name="psum_t", bufs=2, space="PSUM"))
        psum_e = ctx.enter_context(tc.tile_pool(name="psum_e", bufs=2, space="PSUM"))

        gt_sb = consts.tile([128, 3, 32], f32)
        nc.sync.dma_start(gt_sb[:, :, :], gt_d.ap())
        wat_sb = consts.tile([128, 128], f32)
        nc.sync.dma_start(wat_sb[:, :], wat_d.ap())
        vcol_sb = consts.tile([128, 32], f32)
        nc.sync.dma_start(vcol_sb[:, :], vcol_d.ap())

        oh_ap = oh_d.ap()

        def strided4(t, fr=None):
            # view of tile t at partitions {0,32,64,96}: SBUF AP dim0 step is
            # the per-partition address pitch, so stride-32 partitions =
            # 32x the row pitch.
            a = t[:, :] if fr is None else t[:, fr]
            pitch = a.ap[0][0]
            return bass.AP(
                tensor=a.tensor,
                offset=a.offset,
                ap=[[pitch * 32, 4]] + list(a.ap[1:]),
            )

        e_ap = edram.ap()

        def emit_conv_chunk(g, q, hp):
            oh = ohp.tile([128, S, WIN], f32, tag="oh", name="oh")
            for j in range(S):
                src = bass.AP(
                    tensor=oh_ap.tensor,
                    offset=(4 * g) * 32 * L + NL * j + CHUNK * q,
                    ap=[[32 * L, 4], [L, 32], [1, WIN]],
                )
                nc.sync.dma_start(out=oh[:, j, :], in_=src)
            yps = [
                psum_y.tile([128, CHUNK], f32, tag=f"yp{_i}", name=f"yp{_i}")
                for _i in range(4)
            ]
            for k in range(3):
                for j in range(S):
                    for i in range(4):
                        nc.tensor.matmul(
                            out=yps[i][32 * j : 32 * j + 32, :],
                            lhsT=gt_sb[32 * i : 32 * i + 32, k, :],
                            rhs=oh[32 * i : 32 * i + 32, j, k : k + CHUNK],
                            start=(k == 0),
                            stop=(k == 2),
                            skip_group_check=True,
                            tile_position=(32 * i, 32 * j),
                        )
            for i in range(4):
                dst = hp[i][:, CHUNK * q : CHUNK * (q + 1)]
                if i < 3:
                    nc.scalar.activation(dst, yps[i][:, :], AF.Relu)
                else:
                    nc.vector.tensor_scalar_max(dst, yps[i][:, :], 0.0)

        def emit_te_chunk(g, q, hp, tt, ee, zpart):
            for i in range(4):
                tp = psum_t.tile([128, CHUNK], f32, tag="tp", name="tp")
                nc.tensor.matmul(
                    out=tp[:, :],
                    lhsT=wat_sb[:, :],
                    rhs=hp[i][:, CHUNK * q : CHUNK * (q + 1)],
                    start=True,
                    stop=True,
                )
                nc.scalar.activation(
                    tt[i][:, CHUNK * q : CHUNK * (q + 1)], tp[:, :], AF.Tanh
                )
            ep = psum_e.tile([128, CHUNK], f32, tag="ep", name="ep")
            for j in range(4):
                nc.tensor.matmul(
                    out=ep[32 * j : 32 * j + 32, :],
                    lhsT=vcol_sb[:, :],
                    rhs=tt[j][:, CHUNK * q : CHUNK * (q + 1)],
                    start=True,
                    stop=True,
                    tile_position=(0, 32 * j),
                )
            nc.scalar.activation(
                ee[:, CHUNK * q : CHUNK * (q + 1)],
                ep[:, :],
                AF.Exp,
                accum_out=zpart[:, q : q + 1],
            )

        def emit_group_tail(g, ee, zpart):
            zsum_sb = smallp.tile([128, 1], f32, tag="zsum", name="zsum")
            zscr = smallp.tile([128, NCHUNK], f32, tag="zscr", name="zscr")
            nc.scalar.activation(
                zscr[:, :], zpart[:, :], AF.Copy, accum_out=zsum_sb[:, :]
            )
            nc.scalar.dma_start(
                out=zsum_d.ap()[4 * g : 4 * g + 4, :], in_=strided4(zsum_sb)
            )
            dst4 = bass.AP(
                tensor=e_ap.tensor, offset=4 * g * NL, ap=[[NL, 4], [1, NL]]
            )
            nc.scalar.dma_start(out=dst4, in_=strided4(ee))

        ebs = {}

        def emit_eb_prefetch(v):
            g, i = divmod(v, 4)
            eb = scrp.tile([128, NL], f32, tag="eb", name="eb")
            src_b = bass.AP(
                tensor=e_ap.tensor,
                offset=(4 * g + i) * NL,
                ap=[[0, 128], [1, NL]],
            )
            nc.sync.dma_start(out=eb[:, :], in_=src_b)
            ebs[v] = eb

        def emit_ctx_batch(g, i, hp, ctxg):
            eb = ebs.pop(4 * g + i)
            ctxp = smallp.tile([128, NCHUNK], f32, tag="ctxp", name="ctxp")
            for q in range(NCHUNK):
                scr = scrp.tile([128, CHUNK], f32, tag="scr", name="scr")
                nc.vector.tensor_tensor(
                    scr[:, :],
                    hp[i][:, CHUNK * q : CHUNK * (q + 1)],
                    eb[:, CHUNK * q : CHUNK * (q + 1)],
                    ALU.mult,
                )
                scr2 = scrp.tile([128, CHUNK], f32, tag="scr2", name="scr2")
                nc.vector.tensor_scalar(
                    scr2[:, :], scr[:, :], 1.0, None, ALU.mult, ALU.add,
                    accum_out=ctxp[:, q : q + 1],
                )
            scr3 = smallp.tile([128, NCHUNK], f32, tag="scr3", name="scr3")
            nc.vector.tensor_scalar(
                scr3[:, :], ctxp[:, :], 1.0, None, ALU.mult, ALU.add,
                accum_out=ctxg[:, i : i + 1],
            )

        NU = NG * NCHUNK  # 16 conv units
        hps = {}
        tts = {}
        ees = {}
        zparts = {}
        ctxgs = {}

        def unit_conv(u):
            g, q = divmod(u, NCHUNK)
            if q == 0:
                hps[g] = [
                    hpool.tile([128, NL], f32, tag=f"hp{_i}", name=f"hp{_i}")
                    for _i in range(4)
                ]
                ees[g] = eepool.tile([128, NL], f32, tag="ee", name="ee")
                zparts[g] = smallp.tile([128, NCHUNK], f32, tag="zpart", name="zpart")
            emit_conv_chunk(g, q, hps[g])

        def unit_T(u):
            g, q = divmod(u, NCHUNK)
            hp = hps[g]
            tts[(g, q)] = []
            for i in range(4):
                tp = psum_t.tile([128, CHUNK], f32, tag="tp", name="tp")
                nc.tensor.matmul(
                    out=tp[:, :],
                    lhsT=wat_sb[:, :],
                    rhs=hp[i][:, CHUNK * q : CHUNK * (q + 1)],
                    start=True,
                    stop=True,
                )
                ttc = ttpool.tile([128, CHUNK], f32, tag=f"tt{i}", name=f"tt{i}")
                nc.scalar.activation(ttc[:, :], tp[:, :], AF.Tanh)
                tts[(g, q)].append(ttc)

        def unit_e(u):
            g, q = divmod(u, NCHUNK)
            ee, zpart = ees[g], zparts[g]
            ttq = tts.pop((g, q))
            ep = psum_e.tile([128, CHUNK], f32, tag="ep", name="ep")
            for j in range(4):
                nc.tensor.matmul(
                    out=ep[32 * j : 32 * j + 32, :],
                    lhsT=vcol_sb[:, :],
                    rhs=ttq[j][:, :],
                    start=True,
                    stop=True,
                    tile_position=(0, 32 * j),
                )
            nc.scalar.activation(
                ee[:, CHUNK * q : CHUNK * (q + 1)],
                ep[:, :],
                AF.Exp,
                accum_out=zpart[:, q : q + 1],
            )
            if q == NCHUNK - 1:
                emit_group_tail(g, ee, zparts[g])

        def unit_ctx(v):
            g, i = divmod(v, 4)
            if i == 0:
                ctxgs[g] = smallp.tile([128, 4], f32, tag="ctxg", name="ctxg")
            emit_ctx_batch(g, i, hps[g], ctxgs[g])
            if i == 3:
                nc.scalar.dma_start(
                    out=ctxu_d.ap()[:, 4 * g : 4 * g + 4], in_=ctxgs[g][:, :]
                )

        for u in range(NU + 8):
            if u < NU:
                unit_conv(u)
            if 1 <= u < NU + 1:
                unit_T(u - 1)
            if 2 <= u < NU + 2:
                unit_e(u - 2)
            # eb broadcast prefetch at 4g+6+i, ctx compute at 4g+8+i
            if u >= 6 and u - 6 < NG * 4:
                emit_eb_prefetch(u - 6)
            if u >= 8 and u - 8 < NG * 4:
                unit_ctx(u - 8)

    nc.compile()
    _CACHE["prog"] = nc
    return nc


def _host_consts(emb, conv_w, conv_b, W_a, v_a):
    # permutation: d' = s*32 + c  <->  d = c*4 + s
    perm = np.array([(dp % 32) * 4 + dp // 32 for dp in range(128)], dtype=np.int64)
    G = np.einsum("cik,vi->kcv", conv_w.astype(np.float64), emb.astype(np.float64))
    G = G.astype(np.float32)  # [3, 32, 26]
    G[0] += conv_b.astype(np.float32)[:, None]
    gt = np.zeros((128, 3, 32), dtype=np.float32)
    for i in range(4):
        for k in range(3):
            gt[32 * i : 32 * i + 26, k, :] = G[k].T  # [26, 32]
    W_p = W_a[np.ix_(perm, perm)].astype(np.float32)
    wat = np.ascontiguousarray(W_p.T)
    vcol = np.ascontiguousarray(np.repeat(v_a[perm].astype(np.float32)[:, None], 32, axis=1))
    return perm, gt, wat, vcol


def _in_maps(input_seq, emb, conv_w, conv_b, W_a, v_a):
    perm, gt, wat, vcol = _host_consts(emb, conv_w, conv_b, W_a, v_a)
    seq = np.asarray(input_seq).astype(np.int32)
    sym = np.arange(32, dtype=np.int32)[None, :, None]
    maps = []
    for c in range(NCORES):
        ohc = (seq[BPC * c : BPC * (c + 1), None, :] == sym).astype(np.float32)
        maps.append(
            {
                "oh": ohc,
                "gt": gt,
                "wat": wat,
                "vcol": vcol,
            }
        )
    return perm, maps


def _assemble(perm, results):
    out = np.empty((B, D), dtype=np.float32)
    for c, res in enumerate(results):
        ctxu = res["ctxu"]  # [128, 16] d'-major, unnormalized
        zsum = res["zsum"][:, 0]  # [16]
        ctx_p = (ctxu / zsum[None, :]).T  # [16, 128] in d'-space
        blk = out[BPC * c : BPC * (c + 1)]
        blk[:, perm] = ctx_p
    return out


def kernel(input_seq, emb, conv_w, conv_b, W_a, v_a):
    from concourse import bass_utils

    nc = _build_program()
    perm, maps = _in_maps(input_seq, emb, conv_w, conv_b, W_a, v_a)
    res = bass_utils.run_bass_kernel_spmd(nc, maps, core_ids=list(range(NCORES)))
    return _assemble(perm, res.results)


# revision 29
# speedup vs baseline: 1.0124x; 1.0124x over previous
"""Trainium2 Bass kernel for nn_DingoNet_76854144795142.

Pipeline (per core, 16 of 128 batches, pure data parallel):
  seq -> one-hot (26 symbols) -> conv1d as 3 accumulating PE matmuls from
  per-tap tables G_k = conv_w[:,:,k] @ emb.T  (16-way tile_position packing:
  4 batches x 4 sections concurrently on the 128x128 PE array)
  -> relu -> H' (d-major, section-blocked permutation d' = s*32+c)
  -> T = W_a' @ H' (stationary weights) -> tanh (ACT)
  -> e = v' @ tanhT  (M=1 matmuls, 4-way column packing)
  -> exp folded into PSUM evac with accum_out => softmax sums
  -> ctx = H' @ exp(e) via PE ones-broadcast of exp(e) into PSUM +
     fused tensor_tensor_reduce (mult + free-dim accumulate)
  -> normalization + inverse permutation on host.

Weight-only tables (G_k, permuted W_a / v) are precomputed on host.
"""

import os
import sys

import numpy as np

for _p in ("/opt/trn_rl_repo", "/root/.axon_site/_ro/trn_rl_repo"):
    if os.path.isdir(_p) and _p not in sys.path:
        sys.path.insert(0, _p)

# ---- problem constants (hardcoded per task contract) ----
B = 128          # total batch
L = 8194         # sequence length
PL = 8192        # conv output length
S = 4            # sections
NL = 2048        # positions per section
C = 32           # conv channels
V = 26           # vocab
D = 128          # C * S
NCORES = 8
BPC = B // NCORES      # 16 batches per core
NG = BPC // 4          # 4 groups of 4 batches
CHUNK = 512            # free-dim chunk
NCHUNK = NL // CHUNK   # 4 chunks per section
WIN = CHUNK + 2        # window incl. conv taps

_CACHE = {}


def _build_program():
    """Build + compile the Bass/Tile program once per process."""
    if "prog" in _CACHE:
        return _CACHE["prog"]

    from contextlib import ExitStack

    import concourse.bass as bass
    import concourse.tile as tile
    from concourse import bacc, mybir

    f32 = mybir.dt.float32
    i32 = mybir.dt.int32
    i8 = mybir.dt.int8
    bf16 = mybir.dt.bfloat16
    AF = mybir.ActivationFunctionType
    ALU = mybir.AluOpType

    nc = bacc.Bacc(
        "TRN2",
        target_bir_lowering=False,
        debug=False,
        enable_asserts=True,
        num_devices=NCORES,
    )

    oh_d = nc.dram_tensor("oh", [BPC, 32, L], f32, kind="ExternalInput")
    gt_d = nc.dram_tensor("gt", [128, 3, 32], f32, kind="ExternalInput")
    wat_d = nc.dram_tensor("wat", [128, 128], f32, kind="ExternalInput")
    vcol_d = nc.dram_tensor("vcol", [128, 32], f32, kind="ExternalInput")
    ctxu_d = nc.dram_tensor("ctxu", [128, BPC], f32, kind="ExternalOutput")
    edram = nc.dram_tensor("edram", [BPC, NL], f32, kind="Internal")
    zsum_d = nc.dram_tensor("zsum", [BPC, 1], f32, kind="ExternalOutput")

    with tile.TileContext(nc) as tc, ExitStack() as ctx:
        consts = ctx.enter_context(tc.tile_pool(name="consts", bufs=1))
        ohp = ctx.enter_context(tc.tile_pool(name="ohp", bufs=3))
        hpool = ctx.enter_context(tc.tile_pool(name="hpool", bufs=3))
        ttpool = ctx.enter_context(tc.tile_pool(name="ttpool", bufs=3))
        eepool = ctx.enter_context(tc.tile_pool(name="eepool", bufs=2))
        smallp = ctx.enter_context(tc.tile_pool(name="smallp", bufs=2))
        scrp = ctx.enter_context(tc.tile_pool(name="scrp", bufs=2))
        psum_y = ctx.enter_context(tc.tile_pool(name="psum_y", bufs=1, space="PSUM"))
        psum_t = ctx.enter_context(tc.tile_pool(name="psum_t", bufs=3, space="PSUM"))
        psum_e = ctx.enter_context(tc.tile_pool(name="psum_e", bufs=1, space="PSUM"))

        gt_sb = consts.tile([128, 3, 32], f32)
        nc.sync.dma_start(gt_sb[:, :, :], gt_d.ap())
        wat_sb = consts.tile([128, 128], f32)
        nc.sync.dma_start(wat_sb[:, :], wat_d.ap())
        vcol_sb = consts.tile([128, 32], f32)
        nc.sync.dma_start(vcol_sb[:, :], vcol_d.ap())

        oh_ap = oh_d.ap()

        def strided4(t, fr=None):
            # view of tile t at partitions {0,32,64,96}: SBUF AP dim0 step is
            # the per-partition address pitch, so stride-32 partitions =
            # 32x the row pitch.
            a = t[:, :] if fr is None else t[:, fr]
            pitch = a.ap[0][0]
            return bass.AP(
                tensor=a.tensor,
                offset=a.offset,
                ap=[[pitch * 32, 4]] + list(a.ap[1:]),
            )

        e_ap = edram.ap()

        def emit_conv_chunk(g, q, hp):
            oh = ohp.tile([128, S, WIN], f32, tag="oh", name="oh")
            for j in range(S):
                src = bass.AP(
                    tensor=oh_ap.tensor,
                    offset=(4 * g) * 32 * L + NL * j + CHUNK * q,
                    ap=[[32 * L, 4], [L, 32], [1, WIN]],
                )
                nc.sync.dma_start(out=oh[:, j, :], in_=src)
            yps = [
                psum_y.tile([128, CHUNK], f32, tag=f"yp{_i}", name=f"yp{_i}")
                for _i in range(4)
            ]
            for k in range(3):
                for j in range(S):
                    for i in range(4):
                        nc.tensor.matmul(
                            out=yps[i][32 * j : 32 * j + 32, :],
                            lhsT=gt_sb[32 * i : 32 * i + 32, k, :],
                            rhs=oh[32 * i : 32 * i + 32, j, k : k + CHUNK],
                            start=(k == 0),
                            stop=(k == 2),
                            skip_group_check=True,
                            tile_position=(32 * i, 32 * j),
                        )
            for i in range(4):
                dst = hp[i][:, CHUNK * q : CHUNK * (q + 1)]
                if i < 3:
                    nc.scalar.activation(dst, yps[i][:, :], AF.Relu)
                else:
                    nc.vector.tensor_scalar_max(dst, yps[i][:, :], 0.0)

        def emit_te_chunk(g, q, hp, tt, ee, zpart):
            for i in range(4):
                tp = psum_t.tile([128, CHUNK], f32, tag="tp", name="tp")
                nc.tensor.matmul(
                    out=tp[:, :],
                    lhsT=wat_sb[:, :],
                    rhs=hp[i][:, CHUNK * q : CHUNK * (q + 1)],
                    start=True,
                    stop=True,
                )
                nc.scalar.activation(
                    tt[i][:, CHUNK * q : CHUNK * (q + 1)], tp[:, :], AF.Tanh
                )
            ep = psum_e.tile([128, CHUNK], f32, tag="ep", name="ep")
            for j in range(4):
                nc.tensor.matmul(
                    out=ep[32 * j : 32 * j + 32, :],
                    lhsT=vcol_sb[:, :],
                    rhs=tt[j][:, CHUNK * q : CHUNK * (q + 1)],
                    start=True,
                    stop=True,
                    tile_position=(0, 32 * j),
                )
            nc.scalar.activation(
                ee[:, CHUNK * q : CHUNK * (q + 1)],
                ep[:, :],
                AF.Exp,
                accum_out=zpart[:, q : q + 1],
            )

        def emit_group_tail(g, ee, zpart):
            zsum_sb = smallp.tile([128, 1], f32, tag="zsum", name="zsum")
            zscr = smallp.tile([128, NCHUNK], f32, tag="zscr", name="zscr")
            nc.scalar.activation(
                zscr[:, :], zpart[:, :], AF.Copy, accum_out=zsum_sb[:, :]
            )
            nc.scalar.dma_start(
                out=zsum_d.ap()[4 * g : 4 * g + 4, :], in_=strided4(zsum_sb)
            )
            dst4 = bass.AP(
                tensor=e_ap.tensor, offset=4 * g * NL, ap=[[NL, 4], [1, NL]]
            )
            nc.scalar.dma_start(out=dst4, in_=strided4(ee))

        ebs = {}

        def emit_eb_prefetch(v):
            g, i = divmod(v, 4)
            eb = scrp.tile([128, NL], f32, tag="eb", name="eb")
            src_b = bass.AP(
                tensor=e_ap.tensor,
                offset=(4 * g + i) * NL,
                ap=[[0, 128], [1, NL]],
            )
            nc.sync.dma_start(out=eb[:, :], in_=src_b)
            ebs[v] = eb

        def emit_ctx_batch(g, i, hp, ctxg):
            eb = ebs.pop(4 * g + i)
            ctxp = smallp.tile([128, NCHUNK], f32, tag="ctxp", name="ctxp")
            for q in range(NCHUNK):
                scr = scrp.tile([128, CHUNK], f32, tag="scr", name="scr")
                nc.vector.tensor_tensor(
                    scr[:, :],
                    hp[i][:, CHUNK * q : CHUNK * (q + 1)],
                    eb[:, CHUNK * q : CHUNK * (q + 1)],
                    ALU.mult,
                )
                scr2 = scrp.tile([128, CHUNK], f32, tag="scr2", name="scr2")
                nc.vector.tensor_scalar(
                    scr2[:, :], scr[:, :], 1.0, None, ALU.mult, ALU.add,
                    accum_out=ctxp[:, q : q + 1],
                )
            scr3 = smallp.tile([128, NCHUNK], f32, tag="scr3", name="scr3")
            nc.vector.tensor_scalar(
                scr3[:, :], ctxp[:, :], 1.0, None, ALU.mult, ALU.add,
                accum_out=ctxg[:, i : i + 1],
            )

        NU = NG * NCHUNK  # 16 conv units
        hps = {}
        tts = {}
        ees = {}
        zparts = {}
        ctxgs = {}

        def unit_conv(u):
            g, q = divmod(u, NCHUNK)
            if q == 0:
                hps[g] = [
                    hpool.tile([128, NL], f32, tag=f"hp{_i}", name=f"hp{_i}")
                    for _i in range(4)
                ]
                ees[g] = eepool.tile([128, NL], f32, tag="ee", name="ee")
                zparts[g] = smallp.tile([128, NCHUNK], f32, tag="zpart", name="zpart")
            emit_conv_chunk(g, q, hps[g])

        def unit_T(u):
            g, q = divmod(u, NCHUNK)
            hp = hps[g]
            tts[(g, q)] = []
            for i in range(4):
                tp = psum_t.tile([128, CHUNK], f32, tag="tp", name="tp")
                nc.tensor.matmul(
                    out=tp[:, :],
                    lhsT=wat_sb[:, :],
                    rhs=hp[i][:, CHUNK * q : CHUNK * (q + 1)],
                    start=True,
                    stop=True,
                )
                ttc = ttpool.tile([128, CHUNK], f32, tag=f"tt{i}", name=f"tt{i}")
                nc.scalar.activation(ttc[:, :], tp[:, :], AF.Tanh)
                tts[(g, q)].append(ttc)

        def unit_e(u):
            g, q = divmod(u, NCHUNK)
            ee, zpart = ees[g], zparts[g]
            ttq = tts.pop((g, q))
            ep = psum_e.tile([128, CHUNK], f32, tag="ep", name="ep")
            for j in range(4):
                nc.tensor.matmul(
                    out=ep[32 * j : 32 * j + 32, :],
                    lhsT=vcol_sb[:, :],
                    rhs=ttq[j][:, :],
                    start=True,
                    stop=True,
                    tile_position=(0, 32 * j),
                )
            nc.scalar.activation(
                ee[:, CHUNK * q : CHUNK * (q + 1)],
                ep[:, :],
                AF.Exp,
                accum_out=zpart[:, q : q + 1],
            )
            if q == NCHUNK - 1:
                emit_group_tail(g, ee, zparts[g])

        def unit_ctx(v):
            g, i = divmod(v, 4)
            if i == 0:
                ctxgs[g] = smallp.tile([128, 4], f32, tag="ctxg", name="ctxg")
            emit_ctx_batch(g, i, hps[g], ctxgs[g])
            if i == 3:
                nc.scalar.dma_start(
                    out=ctxu_d.ap()[:, 4 * g : 4 * g + 4], in_=ctxgs[g][:, :]
                )

        for u in range(NU + 8):
            if u < NU:
                unit_conv(u)
            if 1 <= u < NU + 1:
                unit_T(u - 1)
            if 2 <= u < NU + 2:
                unit_e(u - 2)
            # eb broadcast prefetch at 4g+6+i, ctx compute at 4g+8+i
            if u >= 6 and u - 6 < NG * 4:
                emit_eb_prefetch(u - 6)
            if u >= 8 and u - 8 < NG * 4:
                unit_ctx(u - 8)

    nc.compile()
    _CACHE["prog"] = nc
    return nc


def _host_consts(emb, conv_w, conv_b, W_a, v_a):
    # permutation: d' = s*32 + c  <->  d = c*4 + s
    perm = np.array([(dp % 32) * 4 + dp // 32 for dp in range(128)], dtype=np.int64)
    G = np.einsum("cik,vi->kcv", conv_w.astype(np.float64), emb.astype(np.float64))
    G = G.astype(np.float32)  # [3, 32, 26]
    G[0] += conv_b.astype(np.float32)[:, None]
    gt = np.zeros((128, 3, 32), dtype=np.float32)
    for i in range(4):
        for k in range(3):
            gt[32 * i : 32 * i + 26, k, :] = G[k].T  # [26, 32]
    W_p = W_a[np.ix_(perm, perm)].astype(np.float32)
    wat = np.ascontiguousarray(W_p.T)
    vcol = np.ascontiguousarray(np.repeat(v_a[perm].astype(np.float32)[:, None], 32, axis=1))
    return perm, gt, wat, vcol


def _in_maps(input_seq, emb, conv_w, conv_b, W_a, v_a):
    perm, gt, wat, vcol = _host_consts(emb, conv_w, conv_b, W_a, v_a)
    seq = np.asarray(input_seq).astype(np.int32)
    sym = np.arange(32, dtype=np.int32)[None, :, None]
    maps = []
    for c in range(NCORES):
        ohc = (seq[BPC * c : BPC * (c + 1), None, :] == sym).astype(np.float32)
        maps.append(
            {
                "oh": ohc,
                "gt": gt,
                "wat": wat,
                "vcol": vcol,
            }
        )
    return perm, maps


def _assemble(perm, results):
    out = np.empty((B, D), dtype=np.float32)
    for c, res in enumerate(results):
        ctxu = res["ctxu"]  # [128, 16] d'-major, unnormalized
        zsum = res["zsum"][:, 0]  # [16]
        ctx_p = (ctxu / zsum[None, :]).T  # [16, 128] in d'-space
        blk = out[BPC * c : BPC * (c + 1)]
        blk[:, perm] = ctx_p
    return out


def kernel(input_seq, emb, conv_w, conv_b, W_a, v_a):
    from concourse import bass_utils

    nc = _build_program()
    perm, maps = _in_maps(input_seq, emb, conv_w, conv_b, W_a, v_a)
    res = bass_utils.run_bass_kernel_spmd(nc, maps, core_ids=list(range(NCORES)))
    return _assemble(perm, res.results)


# revision 30
# speedup vs baseline: 1.0386x; 1.0258x over previous
"""Trainium2 Bass kernel for nn_DingoNet_76854144795142.

Pipeline (per core, 16 of 128 batches, pure data parallel):
  seq -> one-hot (26 symbols) -> conv1d as 3 accumulating PE matmuls from
  per-tap tables G_k = conv_w[:,:,k] @ emb.T  (16-way tile_position packing:
  4 batches x 4 sections concurrently on the 128x128 PE array)
  -> relu -> H' (d-major, section-blocked permutation d' = s*32+c)
  -> T = W_a' @ H' (stationary weights) -> tanh (ACT)
  -> e = v' @ tanhT  (M=1 matmuls, 4-way column packing)
  -> exp folded into PSUM evac with accum_out => softmax sums
  -> ctx = H' @ exp(e) via PE ones-broadcast of exp(e) into PSUM +
     fused tensor_tensor_reduce (mult + free-dim accumulate)
  -> normalization + inverse permutation on host.

Weight-only tables (G_k, permuted W_a / v) are precomputed on host.
"""

import os
import sys

import numpy as np

for _p in ("/opt/trn_rl_repo", "/root/.axon_site/_ro/trn_rl_repo"):
    if os.path.isdir(_p) and _p not in sys.path:
        sys.path.insert(0, _p)

# ---- problem constants (hardcoded per task contract) ----
B = 128          # total batch
L = 8194         # sequence length
PL = 8192        # conv output length
S = 4            # sections
NL = 2048        # positions per section
C = 32           # conv channels
V = 26           # vocab
D = 128          # C * S
NCORES = 8
BPC = B // NCORES      # 16 batches per core
NG = BPC // 4          # 4 groups of 4 batches
CHUNK = 512            # free-dim chunk
NCHUNK = NL // CHUNK   # 4 chunks per section
WIN = CHUNK + 2        # window incl. conv taps

_CACHE = {}


def _build_program():
    """Build + compile the Bass/Tile program once per process."""
    if "prog" in _CACHE:
        return _CACHE["prog"]

    from contextlib import ExitStack

    import concourse.bass as bass
    import concourse.tile as tile
    from concourse import bacc, mybir

    f32 = mybir.dt.float32
    i32 = mybir.dt.int32
    i8 = mybir.dt.int8
    bf16 = mybir.dt.bfloat16
    AF = mybir.ActivationFunctionType
    ALU = mybir.AluOpType

    nc = bacc.Bacc(
        "TRN2",
        target_bir_lowering=False,
        debug=False,
        enable_asserts=True,
        num_devices=NCORES,
    )

    oh_d = nc.dram_tensor("oh", [BPC, 32, L], f32, kind="ExternalInput")
    gt_d = nc.dram_tensor("gt", [128, 3, 32], f32, kind="ExternalInput")
    wat_d = nc.dram_tensor("wat", [128, 128], f32, kind="ExternalInput")
    vcol_d = nc.dram_tensor("vcol", [128, 32], f32, kind="ExternalInput")
    ctxu_d = nc.dram_tensor("ctxu", [128, BPC], f32, kind="ExternalOutput")
    edram = nc.dram_tensor("edram", [BPC, NL], f32, kind="Internal")
    zsum_d = nc.dram_tensor("zsum", [BPC, 1], f32, kind="ExternalOutput")

    with tile.TileContext(nc) as tc, ExitStack() as ctx:
        consts = ctx.enter_context(tc.tile_pool(name="consts", bufs=1))
        ohp = ctx.enter_context(tc.tile_pool(name="ohp", bufs=2))
        hpool = ctx.enter_context(tc.tile_pool(name="hpool", bufs=3))
        ttpool = ctx.enter_context(tc.tile_pool(name="ttpool", bufs=3))
        eepool = ctx.enter_context(tc.tile_pool(name="eepool", bufs=2))
        smallp = ctx.enter_context(tc.tile_pool(name="smallp", bufs=2))
        scrp = ctx.enter_context(tc.tile_pool(name="scrp", bufs=2))
        psum_y = ctx.enter_context(tc.tile_pool(name="psum_y", bufs=1, space="PSUM"))
        psum_t = ctx.enter_context(tc.tile_pool(name="psum_t", bufs=3, space="PSUM"))
        psum_e = ctx.enter_context(tc.tile_pool(name="psum_e", bufs=1, space="PSUM"))

        gt_sb = consts.tile([128, 3, 32], f32)
        nc.sync.dma_start(gt_sb[:, :, :], gt_d.ap())
        wat_sb = consts.tile([128, 128], f32)
        nc.sync.dma_start(wat_sb[:, :], wat_d.ap())
        vcol_sb = consts.tile([128, 32], f32)
        nc.sync.dma_start(vcol_sb[:, :], vcol_d.ap())

        oh_ap = oh_d.ap()

        def strided4(t, fr=None):
            # view of tile t at partitions {0,32,64,96}: SBUF AP dim0 step is
            # the per-partition address pitch, so stride-32 partitions =
            # 32x the row pitch.
            a = t[:, :] if fr is None else t[:, fr]
            pitch = a.ap[0][0]
            return bass.AP(
                tensor=a.tensor,
                offset=a.offset,
                ap=[[pitch * 32, 4]] + list(a.ap[1:]),
            )

        e_ap = edram.ap()

        def emit_conv_chunk(g, q, hp):
            oh = ohp.tile([128, S, WIN], f32, tag="oh", name="oh")
            for j in range(S):
                src = bass.AP(
                    tensor=oh_ap.tensor,
                    offset=(4 * g) * 32 * L + NL * j + CHUNK * q,
                    ap=[[32 * L, 4], [L, 32], [1, WIN]],
                )
                nc.sync.dma_start(out=oh[:, j, :], in_=src)
            yps = [
                psum_y.tile([128, CHUNK], f32, tag=f"yp{_i}", name=f"yp{_i}")
                for _i in range(4)
            ]
            for k in range(3):
                for j in range(S):
                    for i in range(4):
                        nc.tensor.matmul(
                            out=yps[i][32 * j : 32 * j + 32, :],
                            lhsT=gt_sb[32 * i : 32 * i + 32, k, :],
                            rhs=oh[32 * i : 32 * i + 32, j, k : k + CHUNK],
                            start=(k == 0),
                            stop=(k == 2),
                            skip_group_check=True,
                            tile_position=(32 * i, 32 * j),
                        )
            for i in range(4):
                dst = hp[i][:, CHUNK * q : CHUNK * (q + 1)]
                if i < 3:
                    nc.scalar.activation(dst, yps[i][:, :], AF.Relu)
                else:
                    nc.vector.tensor_scalar_max(dst, yps[i][:, :], 0.0)

        def emit_te_chunk(g, q, hp, tt, ee, zpart):
            for i in range(4):
                tp = psum_t.tile([128, CHUNK], f32, tag="tp", name="tp")
                nc.tensor.matmul(
                    out=tp[:, :],
                    lhsT=wat_sb[:, :],
                    rhs=hp[i][:, CHUNK * q : CHUNK * (q + 1)],
                    start=True,
                    stop=True,
                )
                nc.scalar.activation(
                    tt[i][:, CHUNK * q : CHUNK * (q + 1)], tp[:, :], AF.Tanh
                )
            ep = psum_e.tile([128, CHUNK], f32, tag="ep", name="ep")
            for j in range(4):
                nc.tensor.matmul(
                    out=ep[32 * j : 32 * j + 32, :],
                    lhsT=vcol_sb[:, :],
                    rhs=tt[j][:, CHUNK * q : CHUNK * (q + 1)],
                    start=True,
                    stop=True,
                    tile_position=(0, 32 * j),
                )
            nc.scalar.activation(
                ee[:, CHUNK * q : CHUNK * (q + 1)],
                ep[:, :],
                AF.Exp,
                accum_out=zpart[:, q : q + 1],
            )

        def emit_group_tail(g, ee, zpart):
            zsum_sb = smallp.tile([128, 1], f32, tag="zsum", name="zsum")
            zscr = smallp.tile([128, NCHUNK], f32, tag="zscr", name="zscr")
            nc.scalar.activation(
                zscr[:, :], zpart[:, :], AF.Copy, accum_out=zsum_sb[:, :]
            )
            nc.scalar.dma_start(
                out=zsum_d.ap()[4 * g : 4 * g + 4, :], in_=strided4(zsum_sb)
            )
            dst4 = bass.AP(
                tensor=e_ap.tensor, offset=4 * g * NL, ap=[[NL, 4], [1, NL]]
            )
            nc.scalar.dma_start(out=dst4, in_=strided4(ee))

        ebs = {}

        def emit_eb_prefetch(v):
            g, i = divmod(v, 4)
            eb = scrp.tile([128, NL], f32, tag="eb", name="eb")
            src_b = bass.AP(
                tensor=e_ap.tensor,
                offset=(4 * g + i) * NL,
                ap=[[0, 128], [1, NL]],
            )
            nc.sync.dma_start(out=eb[:, :], in_=src_b)
            ebs[v] = eb

        def emit_ctx_batch(g, i, hp, ctxg):
            eb = ebs.pop(4 * g + i)
            ctxp = smallp.tile([128, NCHUNK], f32, tag="ctxp", name="ctxp")
            for q in range(NCHUNK):
                scr = scrp.tile([128, CHUNK], f32, tag="scr", name="scr")
                nc.vector.tensor_tensor(
                    scr[:, :],
                    hp[i][:, CHUNK * q : CHUNK * (q + 1)],
                    eb[:, CHUNK * q : CHUNK * (q + 1)],
                    ALU.mult,
                )
                scr2 = scrp.tile([128, CHUNK], f32, tag="scr2", name="scr2")
                nc.vector.tensor_scalar(
                    scr2[:, :], scr[:, :], 1.0, None, ALU.mult, ALU.add,
                    accum_out=ctxp[:, q : q + 1],
                )
            scr3 = smallp.tile([128, NCHUNK], f32, tag="scr3", name="scr3")
            nc.vector.tensor_scalar(
                scr3[:, :], ctxp[:, :], 1.0, None, ALU.mult, ALU.add,
                accum_out=ctxg[:, i : i + 1],
            )

        NU = NG * NCHUNK  # 16 conv units
        hps = {}
        tts = {}
        ees = {}
        zparts = {}
        ctxgs = {}

        def unit_conv(u):
            g, q = divmod(u, NCHUNK)
            if q == 0:
                hps[g] = [
                    hpool.tile([128, NL], f32, tag=f"hp{_i}", name=f"hp{_i}")
                    for _i in range(4)
                ]
                ees[g] = eepool.tile([128, NL], f32, tag="ee", name="ee")
                zparts[g] = smallp.tile([128, NCHUNK], f32, tag="zpart", name="zpart")
            emit_conv_chunk(g, q, hps[g])

        def unit_T(u):
            g, q = divmod(u, NCHUNK)
            hp = hps[g]
            tts[(g, q)] = []
            for i in range(4):
                tp = psum_t.tile([128, CHUNK], f32, tag="tp", name="tp")
                nc.tensor.matmul(
                    out=tp[:, :],
                    lhsT=wat_sb[:, :],
                    rhs=hp[i][:, CHUNK * q : CHUNK * (q + 1)],
                    start=True,
                    stop=True,
                )
                ttc = ttpool.tile([128, CHUNK], f32, tag=f"tt{i}", name=f"tt{i}")
                nc.scalar.activation(ttc[:, :], tp[:, :], AF.Tanh)
                tts[(g, q)].append(ttc)

        def unit_e(u):
            g, q = divmod(u, NCHUNK)
            ee, zpart = ees[g], zparts[g]
            ttq = tts.pop((g, q))
            ep = psum_e.tile([128, CHUNK], f32, tag="ep", name="ep")
            for j in range(4):
                nc.tensor.matmul(
                    out=ep[32 * j : 32 * j + 32, :],
                    lhsT=vcol_sb[:, :],
                    rhs=ttq[j][:, :],
                    start=True,
                    stop=True,
                    tile_position=(0, 32 * j),
                )
            nc.scalar.activation(
                ee[:, CHUNK * q : CHUNK * (q + 1)],
                ep[:, :],
                AF.Exp,
                accum_out=zpart[:, q : q + 1],
            )
            if q == NCHUNK - 1:
                emit_group_tail(g, ee, zparts[g])

        def unit_ctx(v):
            g, i = divmod(v, 4)
            if i == 0:
                ctxgs[g] = smallp.tile([128, 4], f32, tag="ctxg", name="ctxg")
            emit_ctx_batch(g, i, hps[g], ctxgs[g])
            if i == 3:
                nc.scalar.dma_start(
                    out=ctxu_d.ap()[:, 4 * g : 4 * g + 4], in_=ctxgs[g][:, :]
                )

        for u in range(NU + 8):
            if u < NU:
                unit_conv(u)
            if 1 <= u < NU + 1:
                unit_T(u - 1)
            if 2 <= u < NU + 2:
                unit_e(u - 2)
            # eb broadcast prefetch at 4g+6+i, ctx compute at 4g+8+i
            if u >= 6 and u - 6 < NG * 4:
                emit_eb_prefetch(u - 6)
            if u >= 8 and u - 8 < NG * 4:
                unit_ctx(u - 8)

    nc.compile()
    _CACHE["prog"] = nc
    return nc


def _host_consts(emb, conv_w, conv_b, W_a, v_a):
    # permutation: d' = s*32 + c  <->  d = c*4 + s
    perm = np.array([(dp % 32) * 4 + dp // 32 for dp in range(128)], dtype=np.int64)
    G = np.einsum("cik,vi->kcv", conv_w.astype(np.float64), emb.astype(np.float64))
    G = G.astype(np.float32)  # [3, 32, 26]
    G[0] += conv_b.astype(np.float32)[:, None]
    gt = np.zeros((128, 3, 32), dtype=np.float32)
    for i in range(4):
        for k in range(3):
            gt[32 * i : 32 * i + 26, k, :] = G[k].T  # [26, 32]
    W_p = W_a[np.ix_(perm, perm)].astype(np.float32)
    wat = np.ascontiguousarray(W_p.T)
    vcol = np.ascontiguousarray(np.repeat(v_a[perm].astype(np.float32)[:, None], 32, axis=1))
    return perm, gt, wat, vcol


def _in_maps(input_seq, emb, conv_w, conv_b, W_a, v_a):
    perm, gt, wat, vcol = _host_consts(emb, conv_w, conv_b, W_a, v_a)
    seq = np.asarray(input_seq).astype(np.int32)
    sym = np.arange(32, dtype=np.int32)[None, :, None]
    maps = []
    for c in range(NCORES):
        ohc = (seq[BPC * c : BPC * (c + 1), None, :] == sym).astype(np.float32)
        maps.append(
            {
                "oh": ohc,
                "gt": gt,
                "wat": wat,
                "vcol": vcol,
            }
        )
    return perm, maps


def _assemble(perm, results):
    out = np.empty((B, D), dtype=np.float32)
    for c, res in enumerate(results):
        ctxu = res["ctxu"]  # [128, 16] d'-major, unnormalized
        zsum = res["zsum"][:, 0]  # [16]
        ctx_p = (ctxu / zsum[None, :]).T  # [16, 128] in d'-space
        blk = out[BPC * c : BPC * (c + 1)]
        blk[:, perm] = ctx_p
    return out


def kernel(input_seq, emb, conv_w, conv_b, W_a, v_a):
    from concourse import bass_utils

    nc = _build_program()
    perm, maps = _in_maps(input_seq, emb, conv_w, conv_b, W_a, v_a)
    res = bass_utils.run_bass_kernel_spmd(nc, maps, core_ids=list(range(NCORES)))
    return _assemble(perm, res.results)
